# revision 1
# baseline (speedup 1.0000x reference)
"""MetaConvSmoother Trainium2 kernel (Bass/Tile), data-parallel over 8 NeuronCores.

v3: bf16 intermediate IO to break the DMA-queue bandwidth bottleneck.
  - hypernet MLPs (9 -> 100 -> 147, exact gelu) on PE + ACT in fp32
  - per-sample conv kernels staged as zero-padded bf16 tables in DRAM,
    contiguous per sample: 45 slots x 255 (3 A + 21 S1 + 21 S2),
    U[127-ky] = w[ky, kx]
  - flipped Toeplitz bands loaded in 512-col chunks (round-robin over
    SP/ACT/Pool queues), partition-reversed on the PE with a constant
    anti-diagonal matrix (bf16, exact), prefetched one sample-pair
    ahead (bb bufs=4)
  - each conv stage = banded bf16 matmuls over image rows (lhsT = band
    slices), column taps via free-dim offset reads of the rhs tile,
    fp32 PSUM accumulation:
      Ax   : 3x3, asymmetric pad (top/left 0, bottom/right 1.0)
      tmp_m: 7x7 corr of r = f - Ax        (3 maps)
      G2   : sum_m 7x7 corr of tmp_m
      out  = x + G2          (out stays fp32)
  - x and f are converted to bf16 on the HOST (halves input DMA);
    r and tmp round-trip through DRAM in bf16 (halves the dominant
    store/load traffic)
  - samples processed in pairs with stage interleave so the r/tmp
    store->load latency of s0 hides under s1's matmuls
  - DMA queues: SP = xt/f/xa/r-window loads, ACT = r/tmp/out stores,
    Pool = tmp-window loads + table scatter; band loads round-robin
"""
import numpy as np
import ml_dtypes

import concourse.bass as bass
import concourse.mybir as mybir
from concourse import bacc, bass_utils
from concourse.tile import TileContext

F32 = mybir.dt.float32
F32R = mybir.dt.float32r
BF16 = mybir.dt.bfloat16

S = 8          # samples per core
N = 512
ML = 3
KK = 7
NCORES = 8

# table layout: 128-element slots -> single contiguous run per partition
# (slot windows overlap; bleed only reaches band cols m >= 122+p, never read
#  since M <= 122 everywhere)
TBL = 128
BANDW = 128                     # cols per band slot
NSA = 3                         # A slots per sample
NSS = 42                        # S1+S2 slots per sample
SLOTS = NSA + NSS
ATBL_TOTAL = (S * NSA + 1) * TBL
MTBL_TOTAL = (S * NSS + 1) * TBL
BF = SLOTS * BANDW              # 5760
BFA = NSA * BANDW               # 384
BFS = NSS * BANDW               # 5376

# row tilings (out_row_start, M, input_row_start)
AX_TILES = [(0, 122, -1), (122, 122, 121), (244, 122, 243), (366, 122, 365),
            (488, 24, 487)]
S7_TILES = [(0, 122, -3), (122, 122, 119), (244, 122, 241), (366, 122, 363),
            (488, 24, 485)]


def _sub_ap(base_ap, pattern, offset):
    """Custom access-pattern view: list of [step, count] pairs + elem offset."""
    a = base_ap.copy()
    v = a.ap
    v.clear()
    for p in pattern:
        v.append(list(p))
    a.offset = base_ap.offset + offset
    return a


def _slot_a(kx):
    return kx


def _slot_s1(m, kx):
    return 3 + m * KK + kx


def _slot_s2(m, kx):
    return 24 + m * KK + kx


def build_kernel(nc):
    x = nc.dram_tensor("x", [S, N, N], BF16, kind="ExternalInput").ap()
    f = nc.dram_tensor("f", [S, N, N], BF16, kind="ExternalInput").ap()
    ka = nc.dram_tensor("kernelA", [S, 9], F32, kind="ExternalInput").ap()
    fc_w1 = [nc.dram_tensor(f"fc{i}_w1", [100, 9], F32, kind="ExternalInput").ap()
             for i in (1, 2)]
    fc_b1 = [nc.dram_tensor(f"fc{i}_b1", [100], F32, kind="ExternalInput").ap()
             for i in (1, 2)]
    fc_w2 = [nc.dram_tensor(f"fc{i}_w2", [147, 100], F32, kind="ExternalInput").ap()
             for i in (1, 2)]
    fc_b2 = [nc.dram_tensor(f"fc{i}_b2", [147], F32, kind="ExternalInput").ap()
             for i in (1, 2)]
    out = nc.dram_tensor("out", [S, N, N], F32, kind="ExternalOutput").ap()

    with TileContext(nc) as tc:
        with (
            tc.tile_pool(name="dram", bufs=1, space="DRAM") as dpool,
            tc.tile_pool(name="const", bufs=1) as cpool,
            tc.tile_pool(name="mlp", bufs=1) as mpool,
            tc.tile_pool(name="bfc", bufs=12) as bfpool,
            tc.tile_pool(name="bands", bufs=4) as bpool,
            tc.tile_pool(name="xt", bufs=8) as xt_pool,
            tc.tile_pool(name="xa", bufs=8) as xa_pool,
            tc.tile_pool(name="ft", bufs=6) as ft_pool,
            tc.tile_pool(name="rst", bufs=6) as rst_pool,
            tc.tile_pool(name="rt7", bufs=8) as rhs_pool,
            tc.tile_pool(name="tt", bufs=12) as tt_pool,
            tc.tile_pool(name="tst", bufs=8) as tst_pool,
            tc.tile_pool(name="ob", bufs=6) as ob_pool,
            tc.tile_pool(name="psA", bufs=2, space="PSUM") as psA,
            tc.tile_pool(name="ps1", bufs=2, space="PSUM") as ps1,
            tc.tile_pool(name="ps2", bufs=2, space="PSUM") as ps2,
            tc.tile_pool(name="psx", bufs=2, space="PSUM") as psx,
        ):
            atables = dpool.tile([ATBL_TOTAL], BF16)
            mtables = dpool.tile([MTBL_TOTAL], BF16)
            r_dram = dpool.tile([S, N, N], BF16)
            tmp_dram = dpool.tile([S, ML, N, N], BF16)

            # ---- constants: anti-diagonal reversal matrix Rev[k,p]=d(k+p=127)
            rev0 = cpool.tile([128, 128], F32)
            nc.gpsimd.memset(rev0, 0.0)
            nc.gpsimd.affine_select(
                out=rev0, in_=rev0,
                compare_op=mybir.AluOpType.not_equal,
                fill=1.0, base=-127, pattern=[[1, 128]], channel_multiplier=1)
            rev = cpool.tile([128, 128], BF16, name="revb")
            nc.vector.tensor_copy(rev, rev0)

            # ---- zero-fill tables
            zta = cpool.tile([25, 128], BF16)
            nc.vector.memset(zta, 0.0)
            nc.sync.dma_start(_sub_ap(atables, [[128, 25], [1, 128]], 0), zta)
            ztm = cpool.tile([128, 337], BF16)
            nc.vector.memset(ztm, 0.0)
            nc.sync.dma_start(_sub_ap(mtables, [[337, 128], [1, 337]], 0), ztm)

            # ---------------- MLP + weight staging ----------------
            ident = cpool.tile([128, 128], F32)
            nc.gpsimd.memset(ident, 0.0)
            nc.gpsimd.affine_select(
                out=ident, in_=ident, compare_op=mybir.AluOpType.not_equal,
                fill=1.0, base=0, pattern=[[-1, 128]], channel_multiplier=1)

            vT = mpool.tile([9, S], F32)
            nc.sync.dma_start(vT, ka.rearrange("s k -> k s"))

            w_sb = {}  # (layer i, map m) -> [49, S] conv weights
            for i in range(2):
                w1n = mpool.tile([100, 9], F32, name=f"w1n{i}")
                nc.sync.dma_start(w1n, fc_w1[i])
                W1T = mpool.tile([9, 100], F32, name=f"W1T{i}")
                t1 = psx.tile([9, 100], F32, name=f"t1_{i}", tag="aux")
                nc.tensor.transpose(t1, w1n, ident[:100, :100])
                nc.vector.tensor_copy(W1T, t1)

                b1 = mpool.tile([100, 1], F32, name=f"b1_{i}")
                nc.sync.dma_start(b1, fc_b1[i].unsqueeze(1))

                w2n_a = mpool.tile([128, 100], F32, name=f"w2na{i}")
                nc.sync.dma_start(w2n_a, fc_w2[i][0:128, :])
                w2n_b = mpool.tile([19, 100], F32, name=f"w2nb{i}")
                nc.sync.dma_start(w2n_b, fc_w2[i][128:147, :])
                W2T = mpool.tile([100, 147], F32, name=f"W2T{i}")
                tr_a = psx.tile([100, 128], F32, name=f"tra{i}", tag="aux")
                nc.tensor.transpose(tr_a, w2n_a, ident)
                nc.vector.tensor_copy(W2T[:, 0:128], tr_a)
                tr_b = psx.tile([100, 19], F32, name=f"trb{i}", tag="aux")
                nc.tensor.transpose(tr_b, w2n_b, ident[:19, :19])
                nc.vector.tensor_copy(W2T[:, 128:147], tr_b)

                h_pre = psx.tile([100, S], F32, name=f"hpre{i}", tag="aux")
                nc.tensor.matmul(h_pre, W1T, vT, start=True, stop=True)
                h = mpool.tile([100, S], F32, name=f"h{i}")
                nc.scalar.activation(
                    h, h_pre, mybir.ActivationFunctionType.Gelu, bias=b1)

                for m in range(ML):
                    b2m = mpool.tile([49, 1], F32, name=f"b2_{i}_{m}")
                    nc.sync.dma_start(
                        b2m, fc_b2[i][49 * m:49 * m + 49].unsqueeze(1))
                    wp = psx.tile([49, S], F32, name=f"wp{i}{m}", tag="aux")
                    nc.tensor.matmul(wp, W2T[:, 49 * m:49 * m + 49], h,
                                     start=True, stop=True)
                    wsb = mpool.tile([49, S], F32, name=f"w_{i}_{m}")
                    nc.scalar.activation(
                        wsb, wp, mybir.ActivationFunctionType.Identity,
                        bias=b2m)
                    w_sb[(i, m)] = wsb

            # scatter conv weights into zero-padded tables
            # (SWDGE casts fp32 -> bf16 on the fly)
            # A slots kx=0..2: U[(s*3 + kx)*128 + 127 - ky] = kernelA[s, ky, kx]
            for ky in range(3):
                nc.gpsimd.dma_start(
                    _sub_ap(atables, [[TBL, 3], [NSA * TBL, S]], 127 - ky),
                    vT[3 * ky:3 * ky + 3, :])
            # S slots: U[(s*42 + 21*i + m*7 + kx)*128 + 127 - ky]
            import itertools
            for i, m, ky in itertools.product(range(2), range(ML), range(KK)):
                nc.gpsimd.dma_start(
                    _sub_ap(mtables, [[TBL, KK], [NSS * TBL, S]],
                            (21 * i + m * KK) * TBL + 127 - ky),
                    w_sb[(i, m)][KK * ky:KK * ky + KK, :])

            # ---------------- per-sample emitters ----------------
            bb_of = {}
            dma_q = [nc.sync, nc.scalar, nc.gpsimd]

            def emit_bands(s):
                """Two contiguous band loads (A, S), PE partition-reversal."""
                bb = bpool.tile([128, BF], BF16, name=f"bb{s}", tag="bands")
                bb_of[s] = bb
                # A part: one contiguous-run gather from atables
                bfa = bfpool.tile([128, BFA], BF16, name=f"bfa{s}", tag="bfa")
                nc.sync.dma_start(
                    _sub_ap(bfa, [[BFA, 128], [1, BFA]], 0),
                    _sub_ap(atables, [[1, 128], [1, BFA]], s * NSA * TBL))
                pra = psx.tile([128, 512], F32, name=f"pra{s}", tag="aux")
                nc.tensor.matmul(pra[:, :BFA], rev, bfa,
                                 start=True, stop=True)
                nc.scalar.copy(bb[:, 0:BFA], pra[:, :BFA])
                # S part: 512-col chunks, each one contiguous run/partition
                for i, c in enumerate(range(0, BFS, 512)):
                    w = min(512, BFS - c)
                    bf_c = bfpool.tile([128, 512], BF16, name=f"bfs{s}_{c}",
                                       tag="bfs")
                    dma_q[i % 3].dma_start(
                        _sub_ap(bf_c, [[512, 128], [1, w]], 0),
                        _sub_ap(mtables, [[1, 128], [1, w]],
                                s * NSS * TBL + c))
                    pr = psx.tile([128, 512], F32, name=f"pr{s}_{c}",
                                  tag="aux")
                    nc.tensor.matmul(pr[:, :w], rev, bf_c[:, :w],
                                     start=True, stop=True)
                    nc.scalar.copy(bb[:, BFA + c:BFA + c + w], pr[:, :w])

            def band(s, slot, M):
                bb = bb_of[s]
                return bb[:, slot * BANDW:slot * BANDW + M]

            def emit_ax(s):
                """Ax = corr3(x_pad, A); r = f - Ax -> r_dram (bf16)."""
                for (o0, M, rs) in AX_TILES:
                    xt = xt_pool.tile([128, N + 4], BF16,
                                      name=f"xt{s}_{o0}", tag="xa")
                    if rs + 128 > N:          # bottom tile: ones pad
                        nd = N - rs
                        nc.vector.memset(xt, 1.0)
                        nc.sync.dma_start(xt[0:nd, 2:N + 2], x[s, rs:N, :])
                        nc.vector.memset(xt[0:nd, 0:2], 0.0)
                    else:
                        lo = max(0, rs)
                        p0 = lo - rs
                        if p0 > 0:
                            nc.vector.memset(xt[0:p0, :], 0.0)
                        nc.sync.dma_start(xt[p0:128, 2:N + 2],
                                          x[s, lo:rs + 128, :])
                        nc.vector.memset(xt[:, 0:2], 0.0)
                        nc.vector.memset(xt[:, N + 2:N + 4], 1.0)
                    ps = psA.tile([M, N], F32, name=f"psA{s}_{o0}", tag="ax")
                    for kx in range(3):
                        nc.tensor.matmul(ps, band(s, _slot_a(kx), M),
                                         xt[:, kx + 1:kx + 1 + N],
                                         start=(kx == 0), stop=(kx == 2))
                    ft = ft_pool.tile([122, N], BF16, name=f"ft{s}_{o0}",
                                      tag="f")
                    nc.sync.dma_start(ft[:M, :], f[s, o0:o0 + M, :])
                    rt = rst_pool.tile([122, N], BF16, name=f"rt{s}_{o0}",
                                       tag="r")
                    nc.vector.tensor_sub(rt[:M, :], ft[:M, :], ps)
                    nc.scalar.dma_start(r_dram[s, o0:o0 + M, :], rt[:M, :])

            def emit_s1(s):
                """tmp_m = corr7(r, w1_m) -> tmp_dram (per-map 2D stores)."""
                for ti, (o0, M, rs) in enumerate(S7_TILES):
                    rt7 = rhs_pool.tile([128, N + 8], BF16,
                                        name=f"rt7_{s}_{o0}", tag="rt7")
                    lo = max(0, rs)
                    hi = min(N, rs + 128)
                    if ti == 0 or ti == len(S7_TILES) - 1:
                        nc.vector.memset(rt7, 0.0)
                    else:
                        nc.vector.memset(rt7[:, 0:4], 0.0)
                        nc.vector.memset(rt7[:, N + 4:N + 8], 0.0)
                    nc.sync.dma_start(
                        rt7[lo - rs:hi - rs, 4:N + 4], r_dram[s, lo:hi, :])
                    for m in range(ML):
                        ps = ps1.tile([M, N], F32, name=f"ps1_{s}_{o0}_{m}",
                                      tag="s1")
                        for kx in range(KK):
                            nc.tensor.matmul(ps, band(s, _slot_s1(m, kx), M),
                                             rt7[:, kx + 1:kx + 1 + N],
                                             start=(kx == 0), stop=(kx == 6))
                        tst = tst_pool.tile([122, N], BF16,
                                            name=f"tst{s}_{o0}_{m}", tag="ts")
                        nc.vector.tensor_copy(tst[:M, :], ps)
                        nc.scalar.dma_start(
                            tmp_dram[s, m, o0:o0 + M, :], tst[:M, :])

            def emit_s2(s):
                """G2 = sum_m corr7(tmp_m, w2_m); out = x + G2 (fp32 out)."""
                for ti, (o0, M, rs) in enumerate(S7_TILES):
                    lo = max(0, rs)
                    hi = min(N, rs + 128)
                    tts = []
                    for m in range(ML):
                        tt = tt_pool.tile([128, N + 8], BF16,
                                          name=f"tt{s}_{o0}_{m}", tag="tt")
                        if ti == 0 or ti == len(S7_TILES) - 1:
                            nc.vector.memset(tt, 0.0)
                        else:
                            nc.vector.memset(tt[:, 0:4], 0.0)
                            nc.vector.memset(tt[:, N + 4:N + 8], 0.0)
                        nc.gpsimd.dma_start(
                            tt[lo - rs:hi - rs, 4:N + 4],
                            tmp_dram[s, m, lo:hi, :])
                        tts.append(tt)
                    pg = ps2.tile([M, N], F32, name=f"ps2_{s}_{o0}", tag="s2")
                    idx = 0
                    for m in range(ML):
                        for kx in range(KK):
                            nc.tensor.matmul(
                                pg, band(s, _slot_s2(m, kx), M),
                                tts[m][:, kx + 1:kx + 1 + N],
                                start=(idx == 0), stop=(idx == 20))
                            idx += 1
                    xa = xa_pool.tile([126, N], BF16, name=f"x2_{s}_{o0}",
                                      tag="x2")
                    nc.gpsimd.dma_start(xa[:M, :], x[s, o0:o0 + M, :])
                    ob = ob_pool.tile([126, N], F32, name=f"ob{s}_{o0}",
                                      tag="ob")
                    nc.vector.tensor_add(ob[:M, :], xa[:M, :], pg)
                    nc.sync.dma_start(out[s, o0:o0 + M, :], ob[:M, :])

            # ---------------- main loop: sample pairs ----------------
            emit_bands(0)
            emit_bands(1)
            for pair in range(S // 2):
                s0, s1 = 2 * pair, 2 * pair + 1
                emit_ax(s0)
                emit_ax(s1)
                if s0 + 2 < S:
                    emit_bands(s0 + 2)
                emit_s1(s0)
                emit_s1(s1)
                if s1 + 2 < S:
                    emit_bands(s1 + 2)
                emit_s2(s0)
                emit_s2(s1)
    return nc


_CACHED = None


def _get_nc():
    global _CACHED
    if _CACHED is None:
        nc = bacc.Bacc("TRN2", debug=False, enable_asserts=False,
                       num_devices=NCORES)
        build_kernel(nc)
        nc.compile()
        _CACHED = nc
    return _CACHED


def make_in_maps(x, f, kernelA, fc1_w1, fc1_b1, fc1_w2, fc1_b2,
                 fc2_w1, fc2_b1, fc2_w2, fc2_b2):
    shared = {
        "fc1_w1": np.ascontiguousarray(fc1_w1, np.float32),
        "fc1_b1": np.ascontiguousarray(fc1_b1, np.float32),
        "fc1_w2": np.ascontiguousarray(fc1_w2, np.float32),
        "fc1_b2": np.ascontiguousarray(fc1_b2, np.float32),
        "fc2_w1": np.ascontiguousarray(fc2_w1, np.float32),
        "fc2_b1": np.ascontiguousarray(fc2_b1, np.float32),
        "fc2_w2": np.ascontiguousarray(fc2_w2, np.float32),
        "fc2_b2": np.ascontiguousarray(fc2_b2, np.float32),
    }
    bf = ml_dtypes.bfloat16
    in_maps = []
    for c in range(NCORES):
        sl = slice(S * c, S * (c + 1))
        in_maps.append({
            "x": np.ascontiguousarray(np.asarray(x)[sl, 0].astype(bf)),
            "f": np.ascontiguousarray(np.asarray(f)[sl, 0].astype(bf)),
            "kernelA": np.ascontiguousarray(
                np.asarray(kernelA)[sl, 0].reshape(S, 9), np.float32),
            **shared,
        })
    return in_maps


def kernel(x, f, kernelA, fc1_w1, fc1_b1, fc1_w2, fc1_b2,
           fc2_w1, fc2_b1, fc2_w2, fc2_b2):
    x = np.asarray(x)
    nc = _get_nc()
    in_maps = make_in_maps(x, f, kernelA, fc1_w1, fc1_b1, fc1_w2, fc1_b2,
                           fc2_w1, fc2_b1, fc2_w2, fc2_b2)
    res = bass_utils.run_bass_kernel_spmd(
        nc, in_maps, core_ids=list(range(NCORES)))
    outs = [res.results[c]["out"] for c in range(NCORES)]
    full = np.concatenate(outs, axis=0).reshape(64, 1, N, N).astype(np.float32)
    return full



# revision 18
# speedup vs baseline: 1.4855x; 1.4855x over previous
"""MetaConvSmoother Trainium2 kernel (Bass/Tile), data-parallel over 8 NeuronCores.

v4: fully SBUF-resident pipeline; bands via negative-stride Toeplitz DMA.
  - hypernet MLPs (9 -> 100 -> 147, exact gelu) on PE + ACT in fp32
  - per-sample conv kernels staged as zero-padded bf16 tables in DRAM
    (slot vector v[127-ky] = w[ky,kx]); band matrices materialized by
    DMA reads with partition step -1 (Toeplitz direct, no PE reversal)
  - r and tmp never leave SBUF: conv outputs are evicted from PSUM into
    pre-padded 128-row window tiles (center rows partition-aligned via
    DVE/ACT, 3-row halos moved by tiny SBUF->SBUF DMAs); window bands
    use a 3-segment partition layout (center p=j+ky-3, top halo rows at
    p=122..124, bottom halo rows at p=125..127)
  - last row tile (24 rows) map-packed: S1 writes 3 maps into one psum
    at 32-partition pitch; S2 contracts a stacked [96,518] tmp tile
    against a stacked band (7 streams instead of 21)
  - stage A unchanged: banded 3x3 matmul over padded x tiles,
    asymmetric pad (top/left 0, bottom/right 1.0), r = f - Ax
  - out = x + G2 in fp32
  - DMA queues: sync = xt + band chunk0 + A band, scalar = f + band
    chunk1, vector = out stores, tensor = xa + band chunk2,
    gpsimd(SWDGE) = table scatter + packed bands; spills round-robin
"""
import numpy as np
import ml_dtypes

import concourse.bass as bass
import concourse.mybir as mybir
from concourse import bacc, bass_utils
from concourse.tile import TileContext

F32 = mybir.dt.float32
BF16 = mybir.dt.bfloat16

S = 8          # samples per core
N = 512
ML = 3
KK = 7
NCORES = 8

TBL = 128
NSA = 3                         # A slots per sample
NSS = 42                        # S1+S2 slots per sample
ATBL_TOTAL = (S * NSA + 1) * TBL
MTBL_TOTAL = (S * NSS + 1) * TBL
BFA = NSA * TBL                 # 384
BFS = NSS * TBL                 # 5376
BF = BFA + BFS                  # 5760

# output row tiles (o0, M); windows are rows [122w-3, 122w+124]
OUT_TILES = [(0, 122), (122, 122), (244, 122), (366, 122), (488, 24)]
# stage-A x-input tiles (out_row_start, M, input_row_start = o0-1)
AX_TILES = [(0, 122, -1), (122, 122, 121), (244, 122, 243), (366, 122, 365),
            (488, 24, 487)]
WN = 518                        # window cols: 3 zero | 512 image | 3 zero


def _sub_ap(base_ap, pattern, offset):
    """Custom access-pattern view: list of [step, count] pairs + elem offset."""
    a = base_ap.copy()
    v = a.ap
    v.clear()
    for p in pattern:
        v.append(list(p))
    a.offset = base_ap.offset + offset
    return a


def build_kernel(nc):
    x = nc.dram_tensor("x", [S, N, N], BF16, kind="ExternalInput").ap()
    f = nc.dram_tensor("f", [S, N, N], BF16, kind="ExternalInput").ap()
    ka = nc.dram_tensor("kernelA", [S, 9], F32, kind="ExternalInput").ap()
    fc_w1 = [nc.dram_tensor(f"fc{i}_w1", [100, 9], F32, kind="ExternalInput").ap()
             for i in (1, 2)]
    fc_b1 = [nc.dram_tensor(f"fc{i}_b1", [100], F32, kind="ExternalInput").ap()
             for i in (1, 2)]
    fc_w2 = [nc.dram_tensor(f"fc{i}_w2", [147, 100], F32, kind="ExternalInput").ap()
             for i in (1, 2)]
    fc_b2 = [nc.dram_tensor(f"fc{i}_b2", [147], F32, kind="ExternalInput").ap()
             for i in (1, 2)]
    permS_d = nc.dram_tensor("permS", [128, 128], BF16,
                             kind="ExternalInput").ap()
    permR_d = nc.dram_tensor("permR", [128, 128], BF16,
                             kind="ExternalInput").ap()
    out = nc.dram_tensor("out", [S, N, N], F32, kind="ExternalOutput").ap()

    with TileContext(nc) as tc:
        with (
            tc.tile_pool(name="dram", bufs=1, space="DRAM") as dpool,
            tc.tile_pool(name="const", bufs=1) as cpool,
            tc.tile_pool(name="mlp", bufs=1) as mpool,
            tc.tile_pool(name="bands", bufs=4) as bpool,
            tc.tile_pool(name="hank", bufs=2) as hpool,
            tc.tile_pool(name="sbp", bufs=4) as sbpool,
            tc.tile_pool(name="xt", bufs=10) as xt_pool,
            tc.tile_pool(name="ft", bufs=10) as ft_pool,
            tc.tile_pool(name="rw", bufs=10) as rw_pool,
            tc.tile_pool(name="tw", bufs=24) as tw_pool,
            tc.tile_pool(name="pt", bufs=2) as pt_pool,
            tc.tile_pool(name="st", bufs=2) as st_pool,
            tc.tile_pool(name="xa", bufs=10) as xa_pool,
            tc.tile_pool(name="ob", bufs=6) as ob_pool,
            tc.tile_pool(name="psA", bufs=2, space="PSUM") as psA,
            tc.tile_pool(name="ps1", bufs=2, space="PSUM") as ps1,
            tc.tile_pool(name="ps2", bufs=2, space="PSUM") as ps2,
            tc.tile_pool(name="psx", bufs=2, space="PSUM") as psx,
        ):
            atables = dpool.tile([ATBL_TOTAL], BF16)
            mtables = dpool.tile([MTBL_TOTAL], BF16)

            # ---- zero-fill tables
            zta = cpool.tile([25, 128], BF16)
            nc.vector.memset(zta, 0.0)
            nc.sync.dma_start(_sub_ap(atables, [[128, 25], [1, 128]], 0), zta)
            ztm = cpool.tile([128, 337], BF16)
            nc.vector.memset(ztm, 0.0)
            nc.sync.dma_start(_sub_ap(mtables, [[337, 128], [1, 337]], 0), ztm)

            # ---- host-provided permutation matrices (lhsT for band build)
            # permR[q,p] = 1 iff q = 127-p  (plain reversal: A band, sb2)
            # permS[q,p] = 1 iff q = h(p): 124-p (p<=121), 249-p (122..124),
            #                              127-p (125..127)  (window bands)
            permR = cpool.tile([128, 128], BF16)
            nc.sync.dma_start(permR, permR_d)
            permS = cpool.tile([128, 128], BF16)
            nc.sync.dma_start(permS, permS_d)

            # ---------------- MLP + weight staging ----------------
            ident = cpool.tile([128, 128], F32)
            nc.gpsimd.memset(ident, 0.0)
            nc.gpsimd.affine_select(
                out=ident, in_=ident, compare_op=mybir.AluOpType.not_equal,
                fill=1.0, base=0, pattern=[[-1, 128]], channel_multiplier=1)

            vT = mpool.tile([9, S], F32)
            nc.sync.dma_start(vT, ka.rearrange("s k -> k s"))

            w_sb = {}  # (layer i, map m) -> [49, S] conv weights
            for i in range(2):
                w1n = mpool.tile([100, 9], F32, name=f"w1n{i}")
                nc.sync.dma_start(w1n, fc_w1[i])
                W1T = mpool.tile([9, 100], F32, name=f"W1T{i}")
                t1 = psx.tile([9, 100], F32, name=f"t1_{i}", tag="aux")
                nc.tensor.transpose(t1, w1n, ident[:100, :100])
                nc.vector.tensor_copy(W1T, t1)

                b1 = mpool.tile([100, 1], F32, name=f"b1_{i}")
                nc.sync.dma_start(b1, fc_b1[i].unsqueeze(1))

                w2n_a = mpool.tile([128, 100], F32, name=f"w2na{i}")
                nc.sync.dma_start(w2n_a, fc_w2[i][0:128, :])
                w2n_b = mpool.tile([19, 100], F32, name=f"w2nb{i}")
                nc.sync.dma_start(w2n_b, fc_w2[i][128:147, :])
                W2T = mpool.tile([100, 147], F32, name=f"W2T{i}")
                tr_a = psx.tile([100, 128], F32, name=f"tra{i}", tag="aux")
                nc.tensor.transpose(tr_a, w2n_a, ident)
                nc.vector.tensor_copy(W2T[:, 0:128], tr_a)
                tr_b = psx.tile([100, 19], F32, name=f"trb{i}", tag="aux")
                nc.tensor.transpose(tr_b, w2n_b, ident[:19, :19])
                nc.vector.tensor_copy(W2T[:, 128:147], tr_b)

                h_pre = psx.tile([100, S], F32, name=f"hpre{i}", tag="aux")
                nc.tensor.matmul(h_pre, W1T, vT, start=True, stop=True)
                h = mpool.tile([100, S], F32, name=f"h{i}")
                nc.scalar.activation(
                    h, h_pre, mybir.ActivationFunctionType.Gelu, bias=b1)

                for m in range(ML):
                    b2m = mpool.tile([49, 1], F32, name=f"b2_{i}_{m}")
                    nc.sync.dma_start(
                        b2m, fc_b2[i][49 * m:49 * m + 49].unsqueeze(1))
                    wp = psx.tile([49, S], F32, name=f"wp{i}{m}", tag="aux")
                    nc.tensor.matmul(wp, W2T[:, 49 * m:49 * m + 49], h,
                                     start=True, stop=True)
                    wsb = mpool.tile([49, S], F32, name=f"w_{i}_{m}")
                    nc.scalar.activation(
                        wsb, wp, mybir.ActivationFunctionType.Identity,
                        bias=b2m)
                    w_sb[(i, m)] = wsb

            # scatter conv weights into zero-padded tables
            # (SWDGE casts fp32 -> bf16 on the fly)
            # A slots kx=0..2: U[(s*3 + kx)*128 + 127 - ky] = kernelA[s, ky, kx]
            for ky in range(3):
                nc.gpsimd.dma_start(
                    _sub_ap(atables, [[TBL, 3], [NSA * TBL, S]], 127 - ky),
                    vT[3 * ky:3 * ky + 3, :])
            # S slots: U[(s*42 + 21*i + m*7 + kx)*128 + 127 - ky]
            import itertools
            for i, m, ky in itertools.product(range(2), range(ML), range(KK)):
                nc.gpsimd.dma_start(
                    _sub_ap(mtables, [[TBL, KK], [NSS * TBL, S]],
                            (21 * i + m * KK) * TBL + 127 - ky),
                    w_sb[(i, m)][KK * ky:KK * ky + KK, :])

            # ---------------- per-sample state ----------------
            bb_of = {}
            sb1_of = {}
            sb2_of = {}
            rw_of = {}
            tw_of = {}
            pt_of = {}
            st_of = {}
            hw_q = [nc.sync, nc.scalar]
            spill_rr = [0]

            def spill_q():
                q = hw_q[spill_rr[0] % 2]
                spill_rr[0] += 1
                return q

            def emit_bands(s):
                """Band tiles: Hankel loads (+1 partition step) + PE permute.

                H[q, c] = table[c + q]; band = perm.T @ H gives
                bb[p, c] = table[c + h(p)] for any row permutation h.
                """
                ab = s * NSA * TBL
                mb = s * NSS * TBL
                H = hpool.tile([128, BF], BF16, name=f"H{s}", tag="hank")
                hrow = H.ap[0][0]
                nc.sync.dma_start(
                    H[:, 0:BFA], _sub_ap(atables, [[1, 128], [1, BFA]], ab))
                nc.sync.dma_start(
                    H[:, BFA:BFA + 2688],
                    _sub_ap(mtables, [[1, 128], [1, 2688]], mb))
                nc.scalar.dma_start(
                    H[:, BFA + 2688:BF],
                    _sub_ap(mtables, [[1, 128], [1, 2688]], mb + 2688))

                bb = bpool.tile([128, BF], BF16, name=f"bb{s}", tag="bands")
                bb_of[s] = bb
                pra = psx.tile([128, 512], F32, name=f"pra{s}", tag="aux")
                nc.tensor.matmul(pra[:, :BFA], permR, H[:, 0:BFA],
                                 start=True, stop=True)
                nc.scalar.copy(bb[:, 0:BFA], pra[:, :BFA])
                for c0 in range(0, BFS, 512):
                    w = min(512, BFS - c0)
                    pr = psx.tile([128, 512], F32, name=f"pr{s}_{c0}",
                                  tag="aux")
                    nc.tensor.matmul(pr[:, :w], permS,
                                     H[:, BFA + c0:BFA + c0 + w],
                                     start=True, stop=True)
                    nc.scalar.copy(bb[:, BFA + c0:BFA + c0 + w], pr[:, :w])

                # packed S1-last band: cols kx*96 + m*32 + j
                sb1 = sbpool.tile([128, 672], BF16, name=f"sb1_{s}", tag="sb1")
                sb1_of[s] = sb1
                for kx in range(KK):
                    prk = psx.tile([128, 96], F32, name=f"prk{s}_{kx}",
                                   tag="aux")
                    nc.tensor.matmul(
                        prk, permS,
                        _sub_ap(H, [[hrow, 128], [896, 3], [1, 32]],
                                BFA + kx * 128),
                        start=True, stop=True)
                    nc.scalar.copy(sb1[:, kx * 96:kx * 96 + 96], prk)
                # packed S2-last band: [96 (3m x 32i), 7kx * 24j]
                sb2 = sbpool.tile([96, 168], BF16, name=f"sb2_{s}", tag="sb2")
                sb2_of[s] = sb2
                pp2 = psx.tile([96, 168], F32, name=f"pp2_{s}", tag="aux")
                for m in range(ML):
                    nc.tensor.matmul(
                        pp2[32 * m:32 * m + 32, :], permR[:, 0:32],
                        _sub_ap(H, [[hrow, 128], [128, 7], [1, 24]],
                                BFA + 2688 + m * 896),
                        start=True, stop=True)
                nc.scalar.copy(sb2, pp2)

            def band(s, slot, M):
                bb = bb_of[s]
                return bb[:, BFA + slot * TBL:BFA + slot * TBL + M]

            def band_a(s, kx, M):
                bb = bb_of[s]
                return bb[:, kx * TBL:kx * TBL + M]

            def new_window(pool, nm, tag, w):
                """Acquire a [128, WN] window tile with borders zeroed.

                Partition starts for engine ops must be quadrant-aligned, so
                zero-regions are covered by aligned memsets that later center
                writes / halo-spill DMAs partially overwrite (dep-ordered).
                """
                t = pool.tile([128, WN], BF16, name=nm, tag=tag)
                nc.vector.memset(t[:, 0:3], 0.0)
                nc.vector.memset(t[:, 515:WN], 0.0)
                if w == 0:
                    # top halo rows -3..-1 must be zero
                    nc.vector.memset(t[96:128, 3:515], 0.0)
                elif w == 4:
                    # rows beyond 511 and bottom halo must be zero
                    nc.vector.memset(t[:, 3:515], 0.0)
                return t

            def emit_ax(s):
                """Ax = corr3(x_pad, A); r = f - Ax -> SBUF windows."""
                rws = []
                for w, (o0, M, rs) in enumerate(AX_TILES):
                    xt = xt_pool.tile([128, N + 4], BF16,
                                      name=f"xt{s}_{o0}", tag="xa")
                    if rs + 128 > N:          # bottom tile: ones pad
                        nd = N - rs
                        nc.vector.memset(xt, 1.0)
                        nc.gpsimd.dma_start(xt[0:nd, 2:N + 2], x[s, rs:N, :])
                        nc.vector.memset(xt[0:nd, 0:2], 0.0)
                    else:
                        lo = max(0, rs)
                        p0 = lo - rs
                        if p0 > 0:
                            nc.vector.memset(xt[0:p0, :], 0.0)
                        nc.gpsimd.dma_start(xt[p0:128, 2:N + 2],
                                            x[s, lo:rs + 128, :])
                        nc.vector.memset(xt[:, 0:2], 0.0)
                        nc.vector.memset(xt[:, N + 2:N + 4], 1.0)
                    ps = psA.tile([M, N], F32, name=f"psA{s}_{o0}", tag="ax")
                    for kx in range(3):
                        nc.tensor.matmul(ps, band_a(s, kx, M),
                                         xt[:, kx + 1:kx + 1 + N],
                                         start=(kx == 0), stop=(kx == 2))
                    ft = ft_pool.tile([122, N], BF16, name=f"ft{s}_{o0}",
                                      tag="f")
                    nc.scalar.dma_start(ft[:M, :], f[s, o0:o0 + M, :])
                    rw = new_window(rw_pool, f"rw{s}_{w}", "rw", w)
                    rws.append(rw)
                    # center rows (partition-aligned)
                    nc.vector.tensor_sub(rw[0:M, 3:515], ft[:M, :], ps)
                    if w > 0:
                        # top halo of w from center of w-1; bottom halo of
                        # w-1 from center of w
                        spill_q().dma_start(rw[122:125, 3:515],
                                            rws[w - 1][119:122, 3:515])
                        spill_q().dma_start(rws[w - 1][125:128, 3:515],
                                            rw[0:3, 3:515])
                rw_of[s] = rws

            def emit_s1(s):
                """tmp_m = corr7(r, w1_m) -> SBUF windows (+ packed last)."""
                rws = rw_of[s]
                tws = {m: [] for m in range(ML)}
                for w in range(4):
                    (o0, M) = OUT_TILES[w]
                    for m in range(ML):
                        ps = ps1.tile([M, N], F32, name=f"ps1_{s}_{w}_{m}",
                                      tag="s1")
                        for kx in range(KK):
                            nc.tensor.matmul(ps, band(s, m * KK + kx, M),
                                             rws[w][:, kx:kx + N],
                                             start=(kx == 0), stop=(kx == 6))
                        tw = new_window(tw_pool, f"tw{s}_{w}_{m}", "tw", w)
                        tws[m].append(tw)
                        nc.scalar.copy(tw[0:M, 3:515], ps)
                        if w > 0:
                            spill_q().dma_start(tw[122:125, 3:515],
                                                tws[m][w - 1][119:122, 3:515])
                            spill_q().dma_start(tws[m][w - 1][125:128, 3:515],
                                                tw[0:3, 3:515])
                # last tile: 3 maps packed at 32-partition pitch
                pp = ps1.tile([96, N], F32, name=f"ps1p_{s}", tag="s1")
                sb1 = sb1_of[s]
                for kx in range(KK):
                    nc.tensor.matmul(pp, sb1[:, kx * 96:kx * 96 + 96],
                                     rws[4][:, kx:kx + N],
                                     start=(kx == 0), stop=(kx == 6))
                pt = pt_pool.tile([96, N], BF16, name=f"pt{s}", tag="pt")
                nc.scalar.copy(pt, pp)
                pt_of[s] = pt
                # stacked S2-last rhs: st[32m+i] = tmp_m row 485+i
                st = st_pool.tile([128, WN], BF16, name=f"st{s}", tag="st")
                nc.vector.memset(st, 0.0)
                for m in range(ML):
                    # bottom halo of window 3: tmp rows 488..490
                    spill_q().dma_start(tws[m][3][125:128, 3:515],
                                        pt[32 * m:32 * m + 3, :])
                    # st rows 485..487 from window 3 center
                    spill_q().dma_start(st[32 * m:32 * m + 3, 3:515],
                                        tws[m][3][119:122, 3:515])
                    # st rows 488..511 from pt
                    spill_q().dma_start(st[32 * m + 3:32 * m + 27, 3:515],
                                        pt[32 * m:32 * m + 24, :])
                st_of[s] = st
                tw_of[s] = tws

            def emit_s2(s):
                """G2 = sum_m corr7(tmp_m, w2_m); out = x + G2 (fp32)."""
                tws = tw_of[s]
                for w in range(4):
                    (o0, M) = OUT_TILES[w]
                    pg = ps2.tile([M, N], F32, name=f"ps2_{s}_{w}", tag="s2")
                    idx = 0
                    for m in range(ML):
                        for kx in range(KK):
                            nc.tensor.matmul(
                                pg, band(s, 21 + m * KK + kx, M),
                                tws[m][w][:, kx:kx + N],
                                start=(idx == 0), stop=(idx == 20))
                            idx += 1
                    xa = xa_pool.tile([122, N], BF16, name=f"x2_{s}_{o0}",
                                      tag="x2")
                    nc.gpsimd.dma_start(xa[:M, :], x[s, o0:o0 + M, :])
                    ob = ob_pool.tile([122, N], F32, name=f"ob{s}_{o0}",
                                      tag="ob")
                    nc.vector.tensor_add(ob[:M, :], xa[:M, :], pg)
                    hw_q[w % 2].dma_start(out[s, o0:o0 + M, :], ob[:M, :])
                # last tile: stacked contraction over st
                (o0, M) = OUT_TILES[4]
                sb2 = sb2_of[s]
                st = st_of[s]
                pg = ps2.tile([M, N], F32, name=f"ps2_{s}_4", tag="s2")
                for kx in range(KK):
                    nc.tensor.matmul(pg, sb2[:, kx * 24:kx * 24 + 24],
                                     st[0:96, kx:kx + N],
                                     start=(kx == 0), stop=(kx == 6))
                xa = xa_pool.tile([122, N], BF16, name=f"x2_{s}_{o0}",
                                  tag="x2")
                nc.gpsimd.dma_start(xa[:M, :], x[s, o0:o0 + M, :])
                ob = ob_pool.tile([122, N], F32, name=f"ob{s}_{o0}", tag="ob")
                nc.vector.tensor_add(ob[:M, :], xa[:M, :], pg)
                nc.sync.dma_start(out[s, o0:o0 + M, :], ob[:M, :])

            # ---------------- main loop: sample pairs ----------------
            # bands prefetched a full pair ahead (bb bufs=4); the PE-permute
            # band build for pair k+2 is emitted after pair k's s2 so the
            # bb-slot WAR is already resolved at emission.
            for s in range(4):
                emit_bands(s)
            for pair in range(S // 2):
                s0, s1 = 2 * pair, 2 * pair + 1
                emit_ax(s0)
                emit_ax(s1)
                emit_s1(s0)
                emit_s1(s1)
                emit_s2(s0)
                emit_s2(s1)
                if s0 + 4 < S:
                    emit_bands(s0 + 4)
                if s1 + 4 < S:
                    emit_bands(s1 + 4)
    return nc


_CACHED = None


def _get_nc():
    global _CACHED
    if _CACHED is None:
        nc = bacc.Bacc("TRN2", debug=False, enable_asserts=False,
                       num_devices=NCORES)
        build_kernel(nc)
        nc.compile()
        _CACHED = nc
    return _CACHED


def _perm_mats():
    bf = ml_dtypes.bfloat16
    permR = np.zeros((128, 128), np.float32)
    permS = np.zeros((128, 128), np.float32)
    for p in range(128):
        permR[127 - p, p] = 1.0
        if p <= 121:
            h = 124 - p
        elif p <= 124:
            h = 249 - p
        else:
            h = 127 - p
        permS[h, p] = 1.0
    return permS.astype(bf), permR.astype(bf)


def make_in_maps(x, f, kernelA, fc1_w1, fc1_b1, fc1_w2, fc1_b2,
                 fc2_w1, fc2_b1, fc2_w2, fc2_b2):
    permS, permR = _perm_mats()
    shared = {
        "permS": permS,
        "permR": permR,
        "fc1_w1": np.ascontiguousarray(fc1_w1, np.float32),
        "fc1_b1": np.ascontiguousarray(fc1_b1, np.float32),
        "fc1_w2": np.ascontiguousarray(fc1_w2, np.float32),
        "fc1_b2": np.ascontiguousarray(fc1_b2, np.float32),
        "fc2_w1": np.ascontiguousarray(fc2_w1, np.float32),
        "fc2_b1": np.ascontiguousarray(fc2_b1, np.float32),
        "fc2_w2": np.ascontiguousarray(fc2_w2, np.float32),
        "fc2_b2": np.ascontiguousarray(fc2_b2, np.float32),
    }
    bf = ml_dtypes.bfloat16
    in_maps = []
    for c in range(NCORES):
        sl = slice(S * c, S * (c + 1))
        in_maps.append({
            "x": np.ascontiguousarray(np.asarray(x)[sl, 0].astype(bf)),
            "f": np.ascontiguousarray(np.asarray(f)[sl, 0].astype(bf)),
            "kernelA": np.ascontiguousarray(
                np.asarray(kernelA)[sl, 0].reshape(S, 9), np.float32),
            **shared,
        })
    return in_maps


def kernel(x, f, kernelA, fc1_w1, fc1_b1, fc1_w2, fc1_b2,
           fc2_w1, fc2_b1, fc2_w2, fc2_b2):
    x = np.asarray(x)
    nc = _get_nc()
    in_maps = make_in_maps(x, f, kernelA, fc1_w1, fc1_b1, fc1_w2, fc1_b2,
                           fc2_w1, fc2_b1, fc2_w2, fc2_b2)
    res = bass_utils.run_bass_kernel_spmd(
        nc, in_maps, core_ids=list(range(NCORES)))
    outs = [res.results[c]["out"] for c in range(NCORES)]
    full = np.concatenate(outs, axis=0).reshape(64, 1, N, N).astype(np.float32)
    return full


# revision 24
# speedup vs baseline: 1.5544x; 1.0463x over previous
"""MetaConvSmoother Trainium2 kernel (Bass/Tile), data-parallel over 8 NeuronCores.

v4: fully SBUF-resident pipeline; bands via negative-stride Toeplitz DMA.
  - hypernet MLPs (9 -> 100 -> 147, exact gelu) on PE + ACT in fp32
  - per-sample conv kernels staged as zero-padded bf16 tables in DRAM
    (slot vector v[127-ky] = w[ky,kx]); band matrices materialized by
    DMA reads with partition step -1 (Toeplitz direct, no PE reversal)
  - r and tmp never leave SBUF: conv outputs are evicted from PSUM into
    pre-padded 128-row window tiles (center rows partition-aligned via
    DVE/ACT, 3-row halos moved by tiny SBUF->SBUF DMAs); window bands
    use a 3-segment partition layout (center p=j+ky-3, top halo rows at
    p=122..124, bottom halo rows at p=125..127)
  - last row tile (24 rows) map-packed: S1 writes 3 maps into one psum
    at 32-partition pitch; S2 contracts a stacked [96,518] tmp tile
    against a stacked band (7 streams instead of 21)
  - stage A unchanged: banded 3x3 matmul over padded x tiles,
    asymmetric pad (top/left 0, bottom/right 1.0), r = f - Ax
  - out = x + G2 in fp32
  - DMA queues: sync = xt + band chunk0 + A band, scalar = f + band
    chunk1, vector = out stores, tensor = xa + band chunk2,
    gpsimd(SWDGE) = table scatter + packed bands; spills round-robin
"""
import numpy as np
import ml_dtypes

import concourse.bass as bass
import concourse.mybir as mybir
from concourse import bacc, bass_utils
from concourse.tile import TileContext

F32 = mybir.dt.float32
BF16 = mybir.dt.bfloat16

S = 8          # samples per core
N = 512
ML = 3
KK = 7
NCORES = 8

TBL = 128
NSA = 3                         # A slots per sample
NSS = 42                        # S1+S2 slots per sample
ATBL_TOTAL = (S * NSA + 1) * TBL
MTBL_TOTAL = (S * NSS + 1) * TBL
BFA = NSA * TBL                 # 384
BFS = NSS * TBL                 # 5376
BF = BFA + BFS                  # 5760

# output row tiles (o0, M); windows are rows [122w-3, 122w+124]
OUT_TILES = [(0, 122), (122, 122), (244, 122), (366, 122), (488, 24)]
# stage-A x-input tiles (out_row_start, M, input_row_start = o0-1)
AX_TILES = [(0, 122, -1), (122, 122, 121), (244, 122, 243), (366, 122, 365),
            (488, 24, 487)]
WN = 518                        # window cols: 3 zero | 512 image | 3 zero


def _sub_ap(base_ap, pattern, offset):
    """Custom access-pattern view: list of [step, count] pairs + elem offset."""
    a = base_ap.copy()
    v = a.ap
    v.clear()
    for p in pattern:
        v.append(list(p))
    a.offset = base_ap.offset + offset
    return a


def build_kernel(nc):
    x = nc.dram_tensor("x", [S, N, N], BF16, kind="ExternalInput").ap()
    f = nc.dram_tensor("f", [S, N, N], BF16, kind="ExternalInput").ap()
    ka = nc.dram_tensor("kernelA", [S, 9], F32, kind="ExternalInput").ap()
    fc_w1 = [nc.dram_tensor(f"fc{i}_w1", [100, 9], F32, kind="ExternalInput").ap()
             for i in (1, 2)]
    fc_b1 = [nc.dram_tensor(f"fc{i}_b1", [100], F32, kind="ExternalInput").ap()
             for i in (1, 2)]
    fc_w2 = [nc.dram_tensor(f"fc{i}_w2", [147, 100], F32, kind="ExternalInput").ap()
             for i in (1, 2)]
    fc_b2 = [nc.dram_tensor(f"fc{i}_b2", [147], F32, kind="ExternalInput").ap()
             for i in (1, 2)]
    permS_d = nc.dram_tensor("permS", [128, 128], BF16,
                             kind="ExternalInput").ap()
    permR_d = nc.dram_tensor("permR", [128, 128], BF16,
                             kind="ExternalInput").ap()
    out = nc.dram_tensor("out", [S, N, N], F32, kind="ExternalOutput").ap()

    with TileContext(nc) as tc:
        with (
            tc.tile_pool(name="dram", bufs=1, space="DRAM") as dpool,
            tc.tile_pool(name="const", bufs=1) as cpool,
            tc.tile_pool(name="mlp", bufs=1) as mpool,
            tc.tile_pool(name="bands", bufs=4) as bpool,
            tc.tile_pool(name="hank", bufs=2) as hpool,
            tc.tile_pool(name="sbp", bufs=4) as sbpool,
            tc.tile_pool(name="xt", bufs=10) as xt_pool,
            tc.tile_pool(name="ft", bufs=10) as ft_pool,
            tc.tile_pool(name="rw", bufs=20) as rw_pool,
            tc.tile_pool(name="tw", bufs=24) as tw_pool,
            tc.tile_pool(name="pt", bufs=2) as pt_pool,
            tc.tile_pool(name="st", bufs=2) as st_pool,
            tc.tile_pool(name="xa", bufs=10) as xa_pool,
            tc.tile_pool(name="ob", bufs=6) as ob_pool,
            tc.tile_pool(name="psA", bufs=2, space="PSUM") as psA,
            tc.tile_pool(name="ps1", bufs=2, space="PSUM") as ps1,
            tc.tile_pool(name="ps2", bufs=2, space="PSUM") as ps2,
            tc.tile_pool(name="psx", bufs=2, space="PSUM") as psx,
        ):
            atables = dpool.tile([ATBL_TOTAL], BF16)
            mtables = dpool.tile([MTBL_TOTAL], BF16)

            # ---- zero-fill tables
            zta = cpool.tile([25, 128], BF16)
            nc.vector.memset(zta, 0.0)
            nc.sync.dma_start(_sub_ap(atables, [[128, 25], [1, 128]], 0), zta)
            ztm = cpool.tile([128, 337], BF16)
            nc.vector.memset(ztm, 0.0)
            nc.sync.dma_start(_sub_ap(mtables, [[337, 128], [1, 337]], 0), ztm)

            # ---- host-provided permutation matrices (lhsT for band build)
            # permR[q,p] = 1 iff q = 127-p  (plain reversal: A band, sb2)
            # permS[q,p] = 1 iff q = h(p): 124-p (p<=121), 249-p (122..124),
            #                              127-p (125..127)  (window bands)
            permR = cpool.tile([128, 128], BF16)
            nc.sync.dma_start(permR, permR_d)
            permS = cpool.tile([128, 128], BF16)
            nc.sync.dma_start(permS, permS_d)

            # ---------------- MLP + weight staging ----------------
            ident = cpool.tile([128, 128], F32)
            nc.gpsimd.memset(ident, 0.0)
            nc.gpsimd.affine_select(
                out=ident, in_=ident, compare_op=mybir.AluOpType.not_equal,
                fill=1.0, base=0, pattern=[[-1, 128]], channel_multiplier=1)

            vT = mpool.tile([9, S], F32)
            nc.sync.dma_start(vT, ka.rearrange("s k -> k s"))
            # A-table scatter first: stage A for early samples can start
            # while the hypernet MLP + S-table scatter are still running.
            # A slots kx=0..2: U[(s*3 + kx)*128 + 127 - ky] = kernelA[s, ky, kx]
            for ky in range(3):
                nc.gpsimd.dma_start(
                    _sub_ap(atables, [[TBL, 3], [NSA * TBL, S]], 127 - ky),
                    vT[3 * ky:3 * ky + 3, :])

            w_sb = {}  # (layer i, map m) -> [49, S] conv weights
            h_of = {}
            for i in range(2):
                w1n = mpool.tile([100, 9], F32, name=f"w1n{i}")
                nc.sync.dma_start(w1n, fc_w1[i])
                W1T = mpool.tile([9, 100], F32, name=f"W1T{i}")
                t1 = psx.tile([9, 100], F32, name=f"t1_{i}", tag="aux")
                nc.tensor.transpose(t1, w1n, ident[:100, :100])
                nc.vector.tensor_copy(W1T, t1)

                b1 = mpool.tile([100, 1], F32, name=f"b1_{i}")
                nc.sync.dma_start(b1, fc_b1[i].unsqueeze(1))

                w2n_a = mpool.tile([128, 100], F32, name=f"w2na{i}")
                nc.sync.dma_start(w2n_a, fc_w2[i][0:128, :])
                w2n_b = mpool.tile([19, 100], F32, name=f"w2nb{i}")
                nc.sync.dma_start(w2n_b, fc_w2[i][128:147, :])
                W2T = mpool.tile([100, 147], F32, name=f"W2T{i}")
                tr_a = psx.tile([100, 128], F32, name=f"tra{i}", tag="aux")
                nc.tensor.transpose(tr_a, w2n_a, ident)
                nc.vector.tensor_copy(W2T[:, 0:128], tr_a)
                tr_b = psx.tile([100, 19], F32, name=f"trb{i}", tag="aux")
                nc.tensor.transpose(tr_b, w2n_b, ident[:19, :19])
                nc.vector.tensor_copy(W2T[:, 128:147], tr_b)

                h_pre = psx.tile([100, S], F32, name=f"hpre{i}", tag="aux")
                nc.tensor.matmul(h_pre, W1T, vT, start=True, stop=True)
                h = mpool.tile([100, S], F32, name=f"h{i}")
                nc.scalar.activation(
                    h, h_pre, mybir.ActivationFunctionType.Gelu, bias=b1)
                h_of[i] = (h, W2T)

            # layer-2 matmuls + per-layer scatter, interleaved so the
            # S-table fills as early as possible
            import itertools
            for i, m in itertools.product(range(2), range(ML)):
                h, W2T = h_of[i]
                b2m = mpool.tile([49, 1], F32, name=f"b2_{i}_{m}")
                nc.sync.dma_start(
                    b2m, fc_b2[i][49 * m:49 * m + 49].unsqueeze(1))
                wp = psx.tile([49, S], F32, name=f"wp{i}{m}", tag="aux")
                nc.tensor.matmul(wp, W2T[:, 49 * m:49 * m + 49], h,
                                 start=True, stop=True)
                wsb = mpool.tile([49, S], F32, name=f"w_{i}_{m}")
                nc.scalar.activation(
                    wsb, wp, mybir.ActivationFunctionType.Identity,
                    bias=b2m)
                w_sb[(i, m)] = wsb
                # S slots: U[(s*42 + 21*i + m*7 + kx)*128 + 127 - ky]
                for ky in range(KK):
                    nc.gpsimd.dma_start(
                        _sub_ap(mtables, [[TBL, KK], [NSS * TBL, S]],
                                (21 * i + m * KK) * TBL + 127 - ky),
                        wsb[KK * ky:KK * ky + KK, :])

            # ---------------- per-sample state ----------------
            bb_of = {}
            sb1_of = {}
            sb2_of = {}
            rw_of = {}
            tw_of = {}
            pt_of = {}
            st_of = {}
            hw_q = [nc.sync, nc.scalar]
            spill_rr = [0]

            def spill_q():
                q = hw_q[spill_rr[0] % 2]
                spill_rr[0] += 1
                return q

            def emit_bands_a(s):
                """A-part band: Hankel load (+1 partition step) + PE permute.

                H[q, c] = table[c + q]; band = perm.T @ H gives
                bb[p, c] = table[c + h(p)] for any row permutation h.
                """
                ab = s * NSA * TBL
                ha = hpool.tile([128, BFA], BF16, name=f"ha{s}", tag="hankA",
                                bufs=4)
                nc.sync.dma_start(
                    ha, _sub_ap(atables, [[1, 128], [1, BFA]], ab))
                bb = bpool.tile([128, BF], BF16, name=f"bb{s}", tag="bands")
                bb_of[s] = bb
                pra = psx.tile([128, 512], F32, name=f"pra{s}", tag="aux")
                nc.tensor.matmul(pra[:, :BFA], permR, ha,
                                 start=True, stop=True)
                nc.scalar.copy(bb[:, 0:BFA], pra[:, :BFA])

            def emit_bands_s(s):
                """S-part bands (main windows + packed last-tile bands)."""
                mb = s * NSS * TBL
                bb = bb_of[s]
                hs = hpool.tile([128, BFS], BF16, name=f"hs{s}", tag="hankS",
                                bufs=3)
                hrow = hs.ap[0][0]
                nc.sync.dma_start(
                    hs[:, 0:2688],
                    _sub_ap(mtables, [[1, 128], [1, 2688]], mb))
                nc.scalar.dma_start(
                    hs[:, 2688:BFS],
                    _sub_ap(mtables, [[1, 128], [1, 2688]], mb + 2688))
                for ci, c0 in enumerate(range(0, BFS, 512)):
                    w = min(512, BFS - c0)
                    pr = psx.tile([128, 512], F32, name=f"pr{s}_{c0}",
                                  tag="aux")
                    nc.tensor.matmul(pr[:, :w], permS, hs[:, c0:c0 + w],
                                     start=True, stop=True)
                    if ci % 2 == 0:
                        nc.vector.tensor_copy(bb[:, BFA + c0:BFA + c0 + w],
                                              pr[:, :w])
                    else:
                        nc.scalar.copy(bb[:, BFA + c0:BFA + c0 + w],
                                       pr[:, :w])

                # packed S1-last band: cols kx*96 + m*32 + j
                sb1 = sbpool.tile([128, 672], BF16, name=f"sb1_{s}", tag="sb1")
                sb1_of[s] = sb1
                for kx in range(KK):
                    prk = psx.tile([128, 96], F32, name=f"prk{s}_{kx}",
                                   tag="aux")
                    nc.tensor.matmul(
                        prk, permS,
                        _sub_ap(hs, [[hrow, 128], [896, 3], [1, 32]],
                                kx * 128),
                        start=True, stop=True)
                    nc.scalar.copy(sb1[:, kx * 96:kx * 96 + 96], prk)
                # packed S2-last band: [96 (3m x 32i), 7kx * 24j]
                sb2 = sbpool.tile([96, 168], BF16, name=f"sb2_{s}", tag="sb2")
                sb2_of[s] = sb2
                pp2 = psx.tile([96, 168], F32, name=f"pp2_{s}", tag="aux")
                for m in range(ML):
                    nc.tensor.matmul(
                        pp2[32 * m:32 * m + 32, :], permR[:, 0:32],
                        _sub_ap(hs, [[hrow, 128], [128, 7], [1, 24]],
                                2688 + m * 896),
                        start=True, stop=True)
                nc.scalar.copy(sb2, pp2)

            def emit_bands(s):
                emit_bands_a(s)
                emit_bands_s(s)

            def band(s, slot, M):
                bb = bb_of[s]
                return bb[:, BFA + slot * TBL:BFA + slot * TBL + M]

            def band_a(s, kx, M):
                bb = bb_of[s]
                return bb[:, kx * TBL:kx * TBL + M]

            def new_window(pool, nm, tag, w):
                """Acquire a [128, WN] window tile with borders zeroed.

                Partition starts for engine ops must be quadrant-aligned, so
                zero-regions are covered by aligned memsets that later center
                writes / halo-spill DMAs partially overwrite (dep-ordered).
                """
                t = pool.tile([128, WN], BF16, name=nm, tag=tag)
                nc.vector.memset(t[:, 0:3], 0.0)
                nc.vector.memset(t[:, 515:WN], 0.0)
                if w == 0:
                    # top halo rows -3..-1 must be zero
                    nc.vector.memset(t[96:128, 3:515], 0.0)
                elif w == 4:
                    # rows beyond 511 and bottom halo must be zero
                    nc.vector.memset(t[:, 3:515], 0.0)
                return t

            def emit_ax(s):
                """Ax = corr3(x_pad, A); r = f - Ax -> SBUF windows."""
                rws = []
                for w, (o0, M, rs) in enumerate(AX_TILES):
                    xt = xt_pool.tile([128, N + 4], BF16,
                                      name=f"xt{s}_{o0}", tag="xa")
                    if rs + 128 > N:          # bottom tile: ones pad
                        nd = N - rs
                        nc.vector.memset(xt, 1.0)
                        nc.gpsimd.dma_start(xt[0:nd, 2:N + 2], x[s, rs:N, :])
                        nc.vector.memset(xt[0:nd, 0:2], 0.0)
                    else:
                        lo = max(0, rs)
                        p0 = lo - rs
                        if p0 > 0:
                            nc.vector.memset(xt[0:p0, :], 0.0)
                        nc.gpsimd.dma_start(xt[p0:128, 2:N + 2],
                                            x[s, lo:rs + 128, :])
                        nc.vector.memset(xt[:, 0:2], 0.0)
                        nc.vector.memset(xt[:, N + 2:N + 4], 1.0)
                    ps = psA.tile([M, N], F32, name=f"psA{s}_{o0}", tag="ax")
                    for kx in range(3):
                        nc.tensor.matmul(ps, band_a(s, kx, M),
                                         xt[:, kx + 1:kx + 1 + N],
                                         start=(kx == 0), stop=(kx == 2))
                    ft = ft_pool.tile([122, N], BF16, name=f"ft{s}_{o0}",
                                      tag="f")
                    nc.scalar.dma_start(ft[:M, :], f[s, o0:o0 + M, :])
                    rw = new_window(rw_pool, f"rw{s}_{w}", "rw", w)
                    rws.append(rw)
                    # center rows (partition-aligned)
                    nc.vector.tensor_sub(rw[0:M, 3:515], ft[:M, :], ps)
                    if w > 0:
                        # top halo of w from center of w-1; bottom halo of
                        # w-1 from center of w
                        spill_q().dma_start(rw[122:125, 3:515],
                                            rws[w - 1][119:122, 3:515])
                        spill_q().dma_start(rws[w - 1][125:128, 3:515],
                                            rw[0:3, 3:515])
                rw_of[s] = rws

            def emit_s1(s):
                """tmp_m = corr7(r, w1_m) -> SBUF windows (+ packed last)."""
                rws = rw_of[s]
                tws = {m: [] for m in range(ML)}
                for w in range(4):
                    (o0, M) = OUT_TILES[w]
                    for m in range(ML):
                        ps = ps1.tile([M, N], F32, name=f"ps1_{s}_{w}_{m}",
                                      tag="s1")
                        for kx in range(KK):
                            nc.tensor.matmul(ps, band(s, m * KK + kx, M),
                                             rws[w][:, kx:kx + N],
                                             start=(kx == 0), stop=(kx == 6))
                        tw = new_window(tw_pool, f"tw{s}_{w}_{m}", "tw", w)
                        tws[m].append(tw)
                        if (w + m) % 2 == 0:
                            nc.scalar.copy(tw[0:M, 3:515], ps)
                        else:
                            nc.vector.tensor_copy(tw[0:M, 3:515], ps)
                        if w > 0:
                            spill_q().dma_start(tw[122:125, 3:515],
                                                tws[m][w - 1][119:122, 3:515])
                            spill_q().dma_start(tws[m][w - 1][125:128, 3:515],
                                                tw[0:3, 3:515])
                # last tile: 3 maps packed at 32-partition pitch
                pp = ps1.tile([96, N], F32, name=f"ps1p_{s}", tag="s1")
                sb1 = sb1_of[s]
                for kx in range(KK):
                    nc.tensor.matmul(pp, sb1[:, kx * 96:kx * 96 + 96],
                                     rws[4][:, kx:kx + N],
                                     start=(kx == 0), stop=(kx == 6))
                pt = pt_pool.tile([96, N], BF16, name=f"pt{s}", tag="pt")
                nc.scalar.copy(pt, pp)
                pt_of[s] = pt
                # stacked S2-last rhs: st[32m+i] = tmp_m row 485+i
                st = st_pool.tile([128, WN], BF16, name=f"st{s}", tag="st")
                nc.vector.memset(st, 0.0)
                for m in range(ML):
                    # bottom halo of window 3: tmp rows 488..490
                    spill_q().dma_start(tws[m][3][125:128, 3:515],
                                        pt[32 * m:32 * m + 3, :])
                    # st rows 485..487 from window 3 center
                    spill_q().dma_start(st[32 * m:32 * m + 3, 3:515],
                                        tws[m][3][119:122, 3:515])
                    # st rows 488..511 from pt
                    spill_q().dma_start(st[32 * m + 3:32 * m + 27, 3:515],
                                        pt[32 * m:32 * m + 24, :])
                st_of[s] = st
                tw_of[s] = tws

            def emit_s2(s):
                """G2 = sum_m corr7(tmp_m, w2_m); out = x + G2 (fp32)."""
                tws = tw_of[s]
                for w in range(4):
                    (o0, M) = OUT_TILES[w]
                    pg = ps2.tile([M, N], F32, name=f"ps2_{s}_{w}", tag="s2")
                    idx = 0
                    for m in range(ML):
                        for kx in range(KK):
                            nc.tensor.matmul(
                                pg, band(s, 21 + m * KK + kx, M),
                                tws[m][w][:, kx:kx + N],
                                start=(idx == 0), stop=(idx == 20))
                            idx += 1
                    xa = xa_pool.tile([122, N], BF16, name=f"x2_{s}_{o0}",
                                      tag="x2")
                    nc.gpsimd.dma_start(xa[:M, :], x[s, o0:o0 + M, :])
                    ob = ob_pool.tile([122, N], F32, name=f"ob{s}_{o0}",
                                      tag="ob")
                    nc.vector.tensor_add(ob[:M, :], xa[:M, :], pg)
                    hw_q[w % 2].dma_start(out[s, o0:o0 + M, :], ob[:M, :])
                # last tile: stacked contraction over st
                (o0, M) = OUT_TILES[4]
                sb2 = sb2_of[s]
                st = st_of[s]
                pg = ps2.tile([M, N], F32, name=f"ps2_{s}_4", tag="s2")
                for kx in range(KK):
                    nc.tensor.matmul(pg, sb2[:, kx * 24:kx * 24 + 24],
                                     st[0:96, kx:kx + N],
                                     start=(kx == 0), stop=(kx == 6))
                xa = xa_pool.tile([122, N], BF16, name=f"x2_{s}_{o0}",
                                  tag="x2")
                nc.gpsimd.dma_start(xa[:M, :], x[s, o0:o0 + M, :])
                ob = ob_pool.tile([122, N], F32, name=f"ob{s}_{o0}", tag="ob")
                nc.vector.tensor_add(ob[:M, :], xa[:M, :], pg)
                nc.sync.dma_start(out[s, o0:o0 + M, :], ob[:M, :])

            # ---------------- main loop: sample pairs ----------------
            # Startup: stage A for samples 0..3 runs while the hypernet MLP,
            # S-table scatter, and S-band builds complete. Steady state:
            # bands prefetched a full pair ahead (bb bufs=4); the band build
            # for pair k+2 is emitted after pair k's s2 so the bb-slot WAR
            # is already resolved at emission.
            for s in range(4):
                emit_bands_a(s)
            for s in range(4):
                emit_ax(s)
            for s in range(4):
                emit_bands_s(s)
            for pair in range(S // 2):
                s0, s1 = 2 * pair, 2 * pair + 1
                if pair >= 2:
                    emit_ax(s0)
                    emit_ax(s1)
                emit_s1(s0)
                emit_s1(s1)
                emit_s2(s0)
                emit_s2(s1)
                if s0 + 4 < S:
                    emit_bands(s0 + 4)
                if s1 + 4 < S:
                    emit_bands(s1 + 4)
    return nc


_CACHED = None


def _get_nc():
    global _CACHED
    if _CACHED is None:
        nc = bacc.Bacc("TRN2", debug=False, enable_asserts=False,
                       num_devices=NCORES)
        build_kernel(nc)
        nc.compile()
        _CACHED = nc
    return _CACHED


def _perm_mats():
    bf = ml_dtypes.bfloat16
    permR = np.zeros((128, 128), np.float32)
    permS = np.zeros((128, 128), np.float32)
    for p in range(128):
        permR[127 - p, p] = 1.0
        if p <= 121:
            h = 124 - p
        elif p <= 124:
            h = 249 - p
        else:
            h = 127 - p
        permS[h, p] = 1.0
    return permS.astype(bf), permR.astype(bf)


def make_in_maps(x, f, kernelA, fc1_w1, fc1_b1, fc1_w2, fc1_b2,
                 fc2_w1, fc2_b1, fc2_w2, fc2_b2):
    permS, permR = _perm_mats()
    shared = {
        "permS": permS,
        "permR": permR,
        "fc1_w1": np.ascontiguousarray(fc1_w1, np.float32),
        "fc1_b1": np.ascontiguousarray(fc1_b1, np.float32),
        "fc1_w2": np.ascontiguousarray(fc1_w2, np.float32),
        "fc1_b2": np.ascontiguousarray(fc1_b2, np.float32),
        "fc2_w1": np.ascontiguousarray(fc2_w1, np.float32),
        "fc2_b1": np.ascontiguousarray(fc2_b1, np.float32),
        "fc2_w2": np.ascontiguousarray(fc2_w2, np.float32),
        "fc2_b2": np.ascontiguousarray(fc2_b2, np.float32),
    }
    bf = ml_dtypes.bfloat16
    in_maps = []
    for c in range(NCORES):
        sl = slice(S * c, S * (c + 1))
        in_maps.append({
            "x": np.ascontiguousarray(np.asarray(x)[sl, 0].astype(bf)),
            "f": np.ascontiguousarray(np.asarray(f)[sl, 0].astype(bf)),
            "kernelA": np.ascontiguousarray(
                np.asarray(kernelA)[sl, 0].reshape(S, 9), np.float32),
            **shared,
        })
    return in_maps


def kernel(x, f, kernelA, fc1_w1, fc1_b1, fc1_w2, fc1_b2,
           fc2_w1, fc2_b1, fc2_w2, fc2_b2):
    x = np.asarray(x)
    nc = _get_nc()
    in_maps = make_in_maps(x, f, kernelA, fc1_w1, fc1_b1, fc1_w2, fc1_b2,
                           fc2_w1, fc2_b1, fc2_w2, fc2_b2)
    res = bass_utils.run_bass_kernel_spmd(
        nc, in_maps, core_ids=list(range(NCORES)))
    outs = [res.results[c]["out"] for c in range(NCORES)]
    full = np.concatenate(outs, axis=0).reshape(64, 1, N, N).astype(np.float32)
    return full
